# revision 16
# baseline (speedup 1.0000x reference)
"""Trainium2 Bass kernel for nn_CQAttention (B=24, D=128, N=M=2048), 8 cores.

Data-parallel: 3 batches per core. Per batch (all layouts partition-major):
  Qp[d,m] = Wm[d]*Q[d,m] + Wc[d]                       (DVE, f32r out)
  RT[m,n] = sum_d Qp[d,m] C[d,n] = st[n,m] + sc[n]     (PE, f32r, 1cyc/row)
  sq[m]   = sum_d Wq[d] Q[d,m]                          (PE, fp32, column MMs)
  stripe[m,n] = exp(RT + sq[m] - 60)   bf16, transient per m-tile (ACT)
  rs[m] = rowsum(stripe)  via accum_out;  rsr = 1/rs
  EgS[m,n] = stripe * rsr   fp16  (= S2^T, persistent 2048x2048)
  Aun[d,n] = sum_m QT[m,d] stripe[m,n]                  (PE, fp16 x bf16, phase 1)
  Z1A[p,n] = sum_m rs[m] * EgS[m,n]  (= colsum of EgT)  (PE, RSrep bf16 x fp16)
  EgSR[m,k] = rs[m] * EgS[m,k]  (~EgT), per k-chunk stripe (DVE)
  P[n,k]  = sum_m EgS[m,n] EgSR[m,k]  (symmetric)       (PE, fp16 x bf16)
  Btun[d,k] = sum_n CT[n,d] P[n,k]                      (PE, f32r x f32r)
  out = [C, Aun*rZ, C*Aun*rZ, C*Btun*rZ] * mask         (DVE)
where rZ = 1/Z1A broadcast (Z1A never ~0: worst row max of S is ~2.9 -> e^-57).
Dropout mask (jax key 42) generated host-side in a subprocess, applied on-device.
"""
import os
import subprocess
import sys
import tempfile
from contextlib import ExitStack

import numpy as np
import ml_dtypes

import concourse.bass as bass
import concourse.tile as tile
from concourse import bacc, mybir
from concourse import bass_utils
from concourse import bass_isa

B, D, N = 24, 128, 2048
NCORES, BPC = 8, 3
NT = N // 128          # 16 column-tiles of 128
NCH = N // 512         # 4 chunks of 512
KSH = 60.0             # static softmax shift
DROPOUT_P = 0.1

f32 = mybir.dt.float32
f32r = mybir.dt.float32r
bf16 = mybir.dt.bfloat16
f16 = mybir.dt.float16
Alu = mybir.AluOpType
ExpF = mybir.ActivationFunctionType.Exp


def _body(nc, tc, ctx, C_d, Q_d, W_d, MSK_d, EYE_d, ONES_d, OUT_d, DBG=None):
    const = ctx.enter_context(tc.tile_pool(name="const", bufs=1))
    big = ctx.enter_context(tc.tile_pool(name="big", bufs=1))
    stg = ctx.enter_context(tc.tile_pool(name="stg", bufs=40))
    io = ctx.enter_context(tc.tile_pool(name="io", bufs=1))
    io2 = ctx.enter_context(tc.tile_pool(name="io2", bufs=2))
    small = ctx.enter_context(tc.tile_pool(name="small", bufs=2))
    pnp = ctx.enter_context(tc.tile_pool(name="pnp", bufs=3))
    outp = ctx.enter_context(tc.tile_pool(name="outp", bufs=2))
    mskp = ctx.enter_context(tc.tile_pool(name="mskp", bufs=1))

    EYEt = const.tile([D, D], f32, tag="eye")
    nc.sync.dma_start(EYEt[:], EYE_d)
    ONESb = const.tile([D, D], bf16, tag="ones")
    nc.sync.dma_start(ONESb[:], ONES_d)

    for b in range(BPC):
        ctx.enter_context(nc.named_scope(f"b{b}"))
        # ---------------- prologue ----------------
        Csb = io2.tile([D, N], f32, tag="Csb")
        Qsb = io.tile([D, N], f32, tag="Qsb")
        nc.sync.dma_start(Csb[:], C_d[b])
        nc.sync.dma_start(Qsb[:], Q_d[b])
        Wt = small.tile([D, 3], f32, tag="Wt")      # cols: Wq, Wc, Wm
        nc.sync.dma_start(Wt[:], W_d[b, 0].rearrange("(t d) -> d t", d=D))

        Qp = io.tile([D, N], f32r, tag="s2")        # shared with Ablk
        nc.vector.tensor_scalar(Qp[:], Qsb[:], Wt[:, 2:3], Wt[:, 1:2],
                                Alu.mult, Alu.add)
        Cr = io.tile([D, N], f32r, tag="s3")        # shared with Btb
        nc.vector.tensor_copy(Cr[:], Csb[:])

        EgT = big.tile([D, NT * N], bf16, tag="EgT")   # 64 KB/partition
        rs = small.tile([D, NT], f32, tag="rs")
        rsr = small.tile([D, NT], f32, tag="rsr")

        with tc.tile_pool(name=f"psp_{b}", bufs=1, space="PSUM") as psp:
            # sq[m] as columns of [128, 16] (one zero-region group)
            sqp = psp.tile([D, NT], f32, tag="pro")
            for mt in range(NT):
                nc.tensor.matmul(sqp[:, mt:mt + 1],
                                 Qsb[:, mt * 128:(mt + 1) * 128], Wt[:, 0:1],
                                 start=(mt == 0), stop=(mt == NT - 1))
            sqK = small.tile([D, NT], f32, tag="sqK")
            nc.vector.tensor_scalar_add(sqK[:], sqp[:], -KSH)

            # transposes into one 4-bank tile each; groups of 4 slices per bank
            QTt = io.tile([D, N], f16, tag="QT")
            CTt = io.tile([D, N], f32r, tag="CT")
            tpq = psp.tile([D, N], f32, tag="pro")
            for mt in range(NT):
                sl = slice(mt * 128, (mt + 1) * 128)
                nc.tensor.matmul(tpq[:, sl], Qsb[:, sl], EYEt[:],
                                 is_transpose=True,
                                 start=(mt % 4 == 0), stop=(mt % 4 == 3))
            nc.scalar.copy(QTt[:], tpq[:])
            tpc = psp.tile([D, N], f32, tag="pro")
            for mt in range(NT):
                sl = slice(mt * 128, (mt + 1) * 128)
                nc.tensor.matmul(tpc[:, sl], Csb[:, sl], EYEt[:],
                                 is_transpose=True,
                                 start=(mt % 4 == 0), stop=(mt % 4 == 3))
            nc.vector.tensor_copy(CTt[:], tpc[:])

        # ------- phase 1: RT -> exp into EgT (half-tiles); A accumulation ----
        with tc.tile_pool(name=f"psr_{b}", bufs=2, space="PSUM") as psr, \
             tc.tile_pool(name=f"psa_{b}", bufs=1, space="PSUM") as psa:
            acc_a = psa.tile([D, N], f32, tag="acca")
            for mt in range(NT):
                msl = slice(mt * 128, (mt + 1) * 128)
                rsh = small.tile([D, 2], f32, tag="rsh")
                for h in range(2):
                    rth = psr.tile([D, 1024], f32, tag="rth")
                    for kcl in range(2):
                        kc = h * 2 + kcl
                        nc.tensor.matmul(rth[:, kcl * 512:(kcl + 1) * 512],
                                         Qp[:, msl],
                                         Cr[:, kc * 512:(kc + 1) * 512],
                                         start=True, stop=True)
                    egt_h = EgT[:, mt * N + h * 1024:mt * N + (h + 1) * 1024]
                    nc.scalar.activation(egt_h, rth[:], ExpF,
                                         bias=sqK[:, mt:mt + 1], scale=1.0,
                                         accum_out=rsh[:, h:h + 1])
                    for kcl in range(2):
                        kc = h * 2 + kcl
                        nc.tensor.matmul(
                            acc_a[:, kc * 512:(kc + 1) * 512], QTt[:, msl],
                            egt_h[:, kcl * 512:(kcl + 1) * 512],
                            start=(mt == 0), stop=(mt == NT - 1))
                nc.vector.tensor_add(rs[:, mt:mt + 1], rsh[:, 0:1], rsh[:, 1:2])
                nc.vector.reciprocal(rsr[:, mt:mt + 1], rs[:, mt:mt + 1])

            # ------- phase 2a: A evac (ACT; DVE busy, ACT idle here) -------
            Asb = io.tile([D, N], f32, tag="s1")
            nc.scalar.copy(Asb[:], acc_a[:])

        # ------- phase 2b: Z1 pre-pass, then P (nt-outer) + Bt -------
        Btb = io.tile([D, N], f32, tag="Btb")
        rZ1b = io2.tile([D, N], f32, tag="rZ1b")
        with tc.tile_pool(name=f"psz_{b}", bufs=2, space="PSUM") as psz:
            for kc in range(NCH):
                ksl = slice(kc * 512, (kc + 1) * 512)
                z1c = psz.tile([D, 512], f32, tag="z1c")
                for mt in range(NT):
                    nc.tensor.matmul(
                        z1c[:], ONESb[:],
                        EgT[:, mt * N + kc * 512:mt * N + kc * 512 + 512],
                        start=(mt == 0), stop=(mt == NT - 1))
                nc.vector.reciprocal(rZ1b[:, ksl], z1c[:])

        def stage_est(nt):
            nsl0 = nt * 128
            tiles = []
            for mt in range(NT):
                est_t = stg.tile([D, D], f16, tag="egst")
                nc.vector.tensor_scalar_mul(
                    est_t[:], EgT[:, mt * N + nsl0:mt * N + nsl0 + 128],
                    rsr[:, mt:mt + 1])
                tiles.append(est_t)
            return tiles

        est_q = {0: stage_est(0), 1: stage_est(1)}
        with tc.tile_pool(name=f"pspc_{b}", bufs=4, space="PSUM") as pspc, \
             tc.tile_pool(name=f"psbt_{b}", bufs=1, space="PSUM") as psbt:
            btc = []
            for kc in range(NCH):
                btc_t = psbt.tile([D, 512], f32, tag=f"btc{kc}")
                btc.append(btc_t)
            for nt in range(NT):
                nsl0 = nt * 128
                est = est_q.pop(nt)
                if nt + 2 < NT:
                    est_q[nt + 2] = stage_est(nt + 2)
                pch = []
                for kc in range(NCH):
                    pch_t = pspc.tile([D, 512], f32, tag="pc")
                    pch.append(pch_t)
                for mt in range(NT):
                    for kc in range(NCH):
                        nc.tensor.matmul(
                            pch[kc][:], est[mt][:],
                            EgT[:, mt * N + kc * 512:mt * N + kc * 512 + 512],
                            start=(mt == 0), stop=(mt == NT - 1))
                for kc in range(NCH):
                    pn = pnp.tile([D, 512], f32r, tag="Pn")
                    nc.vector.tensor_copy(pn[:], pch[kc][:])
                    nc.tensor.matmul(btc[kc][:], CTt[:, nsl0:nsl0 + 128],
                                     pn[:],
                                     start=(nt == 0), stop=(nt == NT - 1))
            for kc in range(NCH):
                kslb = slice(kc * 512, (kc + 1) * 512)
                nc.vector.tensor_mul(Btb[:, kslb], btc[kc][:], rZ1b[:, kslb])

        # ---------------- phase 3: outputs ----------------
        if DBG is not None and b == 0:
            nc.sync.dma_start(DBG["Wt"], Wt[:])
            nc.sync.dma_start(DBG["sqK"], sqK[:])
            nc.sync.dma_start(DBG["rs"], rs[:])
            nc.sync.dma_start(DBG["rZ1b"], rZ1b[:])
            nc.sync.dma_start(DBG["Asb"], Asb[:])
            nc.sync.dma_start(DBG["QTt"], QTt[:])
            nc.sync.dma_start(DBG["CTt"], CTt[:].bitcast(mybir.dt.float32))
            nc.sync.dma_start(DBG["EgS0"], EgT[:, 0:N])
        m0 = mskp.tile([D, N], f16, tag="msk")
        nc.sync.dma_start(m0[:], MSK_d[b, 0:D])
        o0 = outp.tile([D, N], f32, tag="ob")
        nc.vector.tensor_mul(o0[:], Csb[:], m0[:])
        nc.sync.dma_start(OUT_d[b, 0:D], o0[:])

        Ablk = Asb
        nc.vector.tensor_mul(Ablk[:], Asb[:], rZ1b[:])
        m1 = mskp.tile([D, N], f16, tag="msk")
        nc.sync.dma_start(m1[:], MSK_d[b, D:2 * D])
        o1 = outp.tile([D, N], f32, tag="ob")
        nc.vector.tensor_mul(o1[:], Ablk[:], m1[:])
        nc.sync.dma_start(OUT_d[b, D:2 * D], o1[:])

        m2 = mskp.tile([D, N], f16, tag="msk")
        nc.sync.dma_start(m2[:], MSK_d[b, 2 * D:3 * D])
        o2 = outp.tile([D, N], f32, tag="ob")
        nc.vector.tensor_mul(o2[:], Csb[:], Ablk[:])
        nc.vector.tensor_mul(o2[:], o2[:], m2[:])
        nc.sync.dma_start(OUT_d[b, 2 * D:3 * D], o2[:])

        m3 = mskp.tile([D, N], f16, tag="msk")
        nc.sync.dma_start(m3[:], MSK_d[b, 3 * D:4 * D])
        o3 = outp.tile([D, N], f32, tag="ob")
        nc.vector.tensor_mul(o3[:], Csb[:], Btb[:])
        nc.vector.tensor_mul(o3[:], o3[:], m3[:])
        nc.sync.dma_start(OUT_d[b, 3 * D:4 * D], o3[:])


_NC_CACHE = {}


def _get_nc():
    if "nc" not in _NC_CACHE:
        nc = bacc.Bacc("TRN2", target_bir_lowering=False, debug=False,
                       num_devices=NCORES)
        C_d = nc.dram_tensor("C", [BPC, D, N], f32, kind="ExternalInput").ap()
        Q_d = nc.dram_tensor("Q", [BPC, D, N], f32, kind="ExternalInput").ap()
        W_d = nc.dram_tensor("W", [BPC, 1, 3 * D], f32, kind="ExternalInput").ap()
        MSK_d = nc.dram_tensor("MSK", [BPC, 4 * D, N], f16,
                               kind="ExternalInput").ap()
        EYE_d = nc.dram_tensor("EYE", [D, D], f32, kind="ExternalInput").ap()
        ONES_d = nc.dram_tensor("ONES", [D, D], bf16, kind="ExternalInput").ap()
        OUT_d = nc.dram_tensor("OUT", [BPC, 4 * D, N], f32,
                               kind="ExternalOutput").ap()
        with tile.TileContext(nc) as tc, ExitStack() as ctx:
            _body(nc, tc, ctx, C_d, Q_d, W_d, MSK_d, EYE_d, OUT_d)
        nc.compile()
        _NC_CACHE["nc"] = nc
    return _NC_CACHE["nc"]


def _gen_mask():
    """Dropout keep-mask from jax key 42, via a CPU-jax subprocess."""
    if "msk" in _NC_CACHE:
        return _NC_CACHE["msk"]
    code = (
        "import jax, numpy as np, sys\n"
        "jax.config.update('jax_platforms', 'cpu')\n"
        "keep = np.asarray(jax.random.bernoulli(jax.random.key(42), "
        f"{1.0 - DROPOUT_P}, ({B}, {4 * D}, {N})))\n"
        "np.save(sys.argv[1], keep)\n"
    )
    with tempfile.NamedTemporaryFile(suffix=".npy", delete=False) as f:
        path = f.name
    env = dict(os.environ)
    env["JAX_PLATFORMS"] = "cpu"
    subprocess.run([sys.executable, "-c", code, path], env=env, check=True,
                   capture_output=True)
    keep = np.load(path)
    os.unlink(path)
    msk = np.where(keep, np.float16(1.0 / (1.0 - DROPOUT_P)),
                   np.float16(0.0)).astype(np.float16)
    _NC_CACHE["msk"] = msk
    return msk


def kernel(C, Q, W):
    C = np.ascontiguousarray(C, dtype=np.float32)
    Q = np.ascontiguousarray(Q, dtype=np.float32)
    W = np.ascontiguousarray(W, dtype=np.float32)
    msk = _gen_mask()
    eye = np.eye(D, dtype=np.float32)
    nc = _get_nc()
    in_maps = []
    for i in range(NCORES):
        sl = slice(i * BPC, (i + 1) * BPC)
        in_maps.append({"C": C[sl], "Q": Q[sl], "W": W[sl], "MSK": msk[sl],
                        "EYE": eye,
                        "ONES": np.ones((D, D), dtype=ml_dtypes.bfloat16)})
    res = bass_utils.run_bass_kernel_spmd(nc, in_maps, core_ids=list(range(NCORES)))
    out = np.concatenate([res.results[i]["OUT"] for i in range(NCORES)], axis=0)
    return out.astype(np.float32)


# revision 18
# speedup vs baseline: 1.0244x; 1.0244x over previous
"""Trainium2 Bass kernel for nn_CQAttention (B=24, D=128, N=M=2048), 8 cores.

Data-parallel: 3 batches per core. Per batch (all layouts partition-major):
  Qp[d,m] = Wm[d]*Q[d,m] + Wc[d]                       (DVE, f32r out)
  RT[m,n] = sum_d Qp[d,m] C[d,n] = st[n,m] + sc[n]     (PE, f32r, 1cyc/row)
  sq[m]   = sum_d Wq[d] Q[d,m]                          (PE, fp32, column MMs)
  stripe[m,n] = exp(RT + sq[m] - 60)   bf16, transient per m-tile (ACT)
  rs[m] = rowsum(stripe)  via accum_out;  rsr = 1/rs
  EgS[m,n] = stripe * rsr   fp16  (= S2^T, persistent 2048x2048)
  Aun[d,n] = sum_m QT[m,d] stripe[m,n]                  (PE, fp16 x bf16, phase 1)
  Z1A[p,n] = sum_m rs[m] * EgS[m,n]  (= colsum of EgT)  (PE, RSrep bf16 x fp16)
  EgSR[m,k] = rs[m] * EgS[m,k]  (~EgT), per k-chunk stripe (DVE)
  P[n,k]  = sum_m EgS[m,n] EgSR[m,k]  (symmetric)       (PE, fp16 x bf16)
  Btun[d,k] = sum_n CT[n,d] P[n,k]                      (PE, f32r x f32r)
  out = [C, Aun*rZ, C*Aun*rZ, C*Btun*rZ] * mask         (DVE)
where rZ = 1/Z1A broadcast (Z1A never ~0: worst row max of S is ~2.9 -> e^-57).
Dropout mask (jax key 42) generated host-side in a subprocess, applied on-device.
"""
import os
import subprocess
import sys
import tempfile
from contextlib import ExitStack

import numpy as np
import ml_dtypes

import concourse.bass as bass
import concourse.tile as tile
from concourse import bacc, mybir
from concourse import bass_utils
from concourse import bass_isa

B, D, N = 24, 128, 2048
NCORES, BPC = 8, 3
NT = N // 128          # 16 column-tiles of 128
NCH = N // 512         # 4 chunks of 512
KSH = 60.0             # static softmax shift
DROPOUT_P = 0.1

f32 = mybir.dt.float32
f32r = mybir.dt.float32r
bf16 = mybir.dt.bfloat16
f16 = mybir.dt.float16
Alu = mybir.AluOpType
ExpF = mybir.ActivationFunctionType.Exp


def _body(nc, tc, ctx, C_d, Q_d, W_d, MSK_d, EYE_d, ONES_d, OUT_d, DBG=None):
    const = ctx.enter_context(tc.tile_pool(name="const", bufs=1))
    big = ctx.enter_context(tc.tile_pool(name="big", bufs=1))
    stg = ctx.enter_context(tc.tile_pool(name="stg", bufs=40))
    io = ctx.enter_context(tc.tile_pool(name="io", bufs=1))
    io2 = ctx.enter_context(tc.tile_pool(name="io2", bufs=2))
    small = ctx.enter_context(tc.tile_pool(name="small", bufs=2))
    pnp = ctx.enter_context(tc.tile_pool(name="pnp", bufs=3))
    outp = ctx.enter_context(tc.tile_pool(name="outp", bufs=2))
    mskp = ctx.enter_context(tc.tile_pool(name="mskp", bufs=1))

    EYEt = const.tile([D, D], f32, tag="eye")
    nc.sync.dma_start(EYEt[:], EYE_d)
    ONESb = const.tile([D, D], bf16, tag="ones")
    nc.sync.dma_start(ONESb[:], ONES_d)

    for b in range(BPC):
        ctx.enter_context(nc.named_scope(f"b{b}"))
        # ---------------- prologue ----------------
        Csb = io2.tile([D, N], f32, tag="Csb")
        Qsb = io.tile([D, N], f32, tag="Qsb")
        nc.sync.dma_start(Csb[:], C_d[b])
        nc.sync.dma_start(Qsb[:], Q_d[b])
        Wt = small.tile([D, 3], f32, tag="Wt")      # cols: Wq, Wc, Wm
        nc.sync.dma_start(Wt[:], W_d[b, 0].rearrange("(t d) -> d t", d=D))

        Qp = io.tile([D, N], f32r, tag="s2")        # shared with Ablk
        nc.vector.tensor_scalar(Qp[:], Qsb[:], Wt[:, 2:3], Wt[:, 1:2],
                                Alu.mult, Alu.add)
        Cr = io.tile([D, N], f32r, tag="s3")        # shared with Btb
        nc.vector.tensor_copy(Cr[:], Csb[:])

        EgT = big.tile([D, NT * N], bf16, tag="EgT")   # 64 KB/partition
        rs = small.tile([D, NT], f32, tag="rs")
        rsr = small.tile([D, NT], f32, tag="rsr")

        with tc.tile_pool(name=f"psp_{b}", bufs=1, space="PSUM") as psp:
            # sq[m] as columns of [128, 16] (one zero-region group)
            sqp = psp.tile([D, NT], f32, tag="pro")
            for mt in range(NT):
                nc.tensor.matmul(sqp[:, mt:mt + 1],
                                 Qsb[:, mt * 128:(mt + 1) * 128], Wt[:, 0:1],
                                 start=(mt == 0), stop=(mt == NT - 1))
            sqK = small.tile([D, NT], f32, tag="sqK")
            nc.vector.tensor_scalar_add(sqK[:], sqp[:], -KSH)

            # transposes into one 4-bank tile each; groups of 4 slices per bank
            QTt = io.tile([D, N], f16, tag="QT")
            CTt = io.tile([D, N], f32r, tag="CT")
            tpq = psp.tile([D, N], f32, tag="pro")
            for mt in range(NT):
                sl = slice(mt * 128, (mt + 1) * 128)
                nc.tensor.matmul(tpq[:, sl], Qsb[:, sl], EYEt[:],
                                 is_transpose=True,
                                 start=(mt % 4 == 0), stop=(mt % 4 == 3))
            nc.scalar.copy(QTt[:], tpq[:])
            tpc = psp.tile([D, N], f32, tag="pro")
            for mt in range(NT):
                sl = slice(mt * 128, (mt + 1) * 128)
                nc.tensor.matmul(tpc[:, sl], Csb[:, sl], EYEt[:],
                                 is_transpose=True,
                                 start=(mt % 4 == 0), stop=(mt % 4 == 3))
            nc.vector.tensor_copy(CTt[:], tpc[:])

        # ------- phase 1: RT -> exp into EgT (half-tiles); A accumulation ----
        with tc.tile_pool(name=f"psr_{b}", bufs=2, space="PSUM") as psr, \
             tc.tile_pool(name=f"psa_{b}", bufs=1, space="PSUM") as psa:
            acc_a = psa.tile([D, N], f32, tag="acca")
            for mt in range(NT):
                msl = slice(mt * 128, (mt + 1) * 128)
                rsh = small.tile([D, 2], f32, tag="rsh")
                for h in range(2):
                    rth = psr.tile([D, 1024], f32, tag="rth")
                    for kcl in range(2):
                        kc = h * 2 + kcl
                        nc.tensor.matmul(rth[:, kcl * 512:(kcl + 1) * 512],
                                         Qp[:, msl],
                                         Cr[:, kc * 512:(kc + 1) * 512],
                                         start=True, stop=True)
                    egt_h = EgT[:, mt * N + h * 1024:mt * N + (h + 1) * 1024]
                    nc.scalar.activation(egt_h, rth[:], ExpF,
                                         bias=sqK[:, mt:mt + 1], scale=1.0,
                                         accum_out=rsh[:, h:h + 1])
                    for kcl in range(2):
                        kc = h * 2 + kcl
                        nc.tensor.matmul(
                            acc_a[:, kc * 512:(kc + 1) * 512], QTt[:, msl],
                            egt_h[:, kcl * 512:(kcl + 1) * 512],
                            start=(mt == 0), stop=(mt == NT - 1))
                nc.vector.tensor_add(rs[:, mt:mt + 1], rsh[:, 0:1], rsh[:, 1:2])
                nc.vector.reciprocal(rsr[:, mt:mt + 1], rs[:, mt:mt + 1])

            # ------- phase 2a: A evac (ACT); Z1 pre-pass on rth slots -------
            Asb = io.tile([D, N], f32, tag="s1")
            nc.scalar.copy(Asb[:], acc_a[:])
            rZ1b = io2.tile([D, N], f32, tag="rZ1b")
            for kc in range(NCH):
                ksl = slice(kc * 512, (kc + 1) * 512)
                z1c = psr.tile([D, 512], f32, tag="rth")
                for mt in range(NT):
                    nc.tensor.matmul(
                        z1c[:], ONESb[:],
                        EgT[:, mt * N + kc * 512:mt * N + kc * 512 + 512],
                        start=(mt == 0), stop=(mt == NT - 1))
                nc.vector.reciprocal(rZ1b[:, ksl], z1c[:])

        # ------- phase 2b: P (nt-outer) + Bt -------
        Btb = io.tile([D, N], f32, tag="Btb")

        def stage_est(nt):
            nsl0 = nt * 128
            tiles = []
            for mt in range(NT):
                est_t = stg.tile([D, D], f16, tag="egst")
                nc.vector.tensor_scalar_mul(
                    est_t[:], EgT[:, mt * N + nsl0:mt * N + nsl0 + 128],
                    rsr[:, mt:mt + 1])
                tiles.append(est_t)
            return tiles

        est_q = {0: stage_est(0), 1: stage_est(1)}
        with tc.tile_pool(name=f"pspc_{b}", bufs=4, space="PSUM") as pspc, \
             tc.tile_pool(name=f"psbt_{b}", bufs=1, space="PSUM") as psbt:
            btc = []
            for kc in range(NCH):
                btc_t = psbt.tile([D, 512], f32, tag=f"btc{kc}")
                btc.append(btc_t)
            for nt in range(NT):
                nsl0 = nt * 128
                est = est_q.pop(nt)
                if nt + 2 < NT:
                    est_q[nt + 2] = stage_est(nt + 2)
                pns = []
                for pr in range(2):
                    pch = []
                    for kcl in range(2):
                        pch_t = pspc.tile([D, 512], f32, tag="pc")
                        pch.append(pch_t)
                    for mt in range(NT):
                        for kcl in range(2):
                            kc = pr * 2 + kcl
                            nc.tensor.matmul(
                                pch[kcl][:], est[mt][:],
                                EgT[:, mt * N + kc * 512:mt * N + kc * 512 + 512],
                                start=(mt == 0), stop=(mt == NT - 1))
                    for kcl in range(2):
                        kc = pr * 2 + kcl
                        pn = pnp.tile([D, 512], f32r, tag="Pn")
                        nc.vector.tensor_copy(pn[:], pch[kcl][:])
                        pns.append((kc, pn))
                for kc, pn in pns:
                    nc.tensor.matmul(btc[kc][:], CTt[:, nsl0:nsl0 + 128],
                                     pn[:],
                                     start=(nt == 0), stop=(nt == NT - 1))
            for kc in range(NCH):
                kslb = slice(kc * 512, (kc + 1) * 512)
                nc.vector.tensor_mul(Btb[:, kslb], btc[kc][:], rZ1b[:, kslb])

        # ---------------- phase 3: outputs ----------------
        if DBG is not None and b == 0:
            nc.sync.dma_start(DBG["Wt"], Wt[:])
            nc.sync.dma_start(DBG["sqK"], sqK[:])
            nc.sync.dma_start(DBG["rs"], rs[:])
            nc.sync.dma_start(DBG["rZ1b"], rZ1b[:])
            nc.sync.dma_start(DBG["Asb"], Asb[:])
            nc.sync.dma_start(DBG["QTt"], QTt[:])
            nc.sync.dma_start(DBG["CTt"], CTt[:].bitcast(mybir.dt.float32))
            nc.sync.dma_start(DBG["EgS0"], EgT[:, 0:N])
        m0 = mskp.tile([D, N], f16, tag="msk")
        nc.sync.dma_start(m0[:], MSK_d[b, 0:D])
        o0 = outp.tile([D, N], f32, tag="ob")
        nc.vector.tensor_mul(o0[:], Csb[:], m0[:])
        nc.sync.dma_start(OUT_d[b, 0:D], o0[:])

        Ablk = Asb
        nc.vector.tensor_mul(Ablk[:], Asb[:], rZ1b[:])
        m1 = mskp.tile([D, N], f16, tag="msk")
        nc.sync.dma_start(m1[:], MSK_d[b, D:2 * D])
        o1 = outp.tile([D, N], f32, tag="ob")
        nc.vector.tensor_mul(o1[:], Ablk[:], m1[:])
        nc.sync.dma_start(OUT_d[b, D:2 * D], o1[:])

        m2 = mskp.tile([D, N], f16, tag="msk")
        nc.sync.dma_start(m2[:], MSK_d[b, 2 * D:3 * D])
        o2 = outp.tile([D, N], f32, tag="ob")
        nc.vector.tensor_mul(o2[:], Csb[:], Ablk[:])
        nc.vector.tensor_mul(o2[:], o2[:], m2[:])
        nc.sync.dma_start(OUT_d[b, 2 * D:3 * D], o2[:])

        m3 = mskp.tile([D, N], f16, tag="msk")
        nc.sync.dma_start(m3[:], MSK_d[b, 3 * D:4 * D])
        o3 = outp.tile([D, N], f32, tag="ob")
        nc.vector.tensor_mul(o3[:], Csb[:], Btb[:])
        nc.vector.tensor_mul(o3[:], o3[:], m3[:])
        nc.sync.dma_start(OUT_d[b, 3 * D:4 * D], o3[:])


_NC_CACHE = {}


def _get_nc():
    if "nc" not in _NC_CACHE:
        nc = bacc.Bacc("TRN2", target_bir_lowering=False, debug=False,
                       num_devices=NCORES)
        C_d = nc.dram_tensor("C", [BPC, D, N], f32, kind="ExternalInput").ap()
        Q_d = nc.dram_tensor("Q", [BPC, D, N], f32, kind="ExternalInput").ap()
        W_d = nc.dram_tensor("W", [BPC, 1, 3 * D], f32, kind="ExternalInput").ap()
        MSK_d = nc.dram_tensor("MSK", [BPC, 4 * D, N], f16,
                               kind="ExternalInput").ap()
        EYE_d = nc.dram_tensor("EYE", [D, D], f32, kind="ExternalInput").ap()
        ONES_d = nc.dram_tensor("ONES", [D, D], bf16, kind="ExternalInput").ap()
        OUT_d = nc.dram_tensor("OUT", [BPC, 4 * D, N], f32,
                               kind="ExternalOutput").ap()
        with tile.TileContext(nc) as tc, ExitStack() as ctx:
            _body(nc, tc, ctx, C_d, Q_d, W_d, MSK_d, EYE_d, OUT_d)
        nc.compile()
        _NC_CACHE["nc"] = nc
    return _NC_CACHE["nc"]


def _gen_mask():
    """Dropout keep-mask from jax key 42, via a CPU-jax subprocess."""
    if "msk" in _NC_CACHE:
        return _NC_CACHE["msk"]
    code = (
        "import jax, numpy as np, sys\n"
        "jax.config.update('jax_platforms', 'cpu')\n"
        "keep = np.asarray(jax.random.bernoulli(jax.random.key(42), "
        f"{1.0 - DROPOUT_P}, ({B}, {4 * D}, {N})))\n"
        "np.save(sys.argv[1], keep)\n"
    )
    with tempfile.NamedTemporaryFile(suffix=".npy", delete=False) as f:
        path = f.name
    env = dict(os.environ)
    env["JAX_PLATFORMS"] = "cpu"
    subprocess.run([sys.executable, "-c", code, path], env=env, check=True,
                   capture_output=True)
    keep = np.load(path)
    os.unlink(path)
    msk = np.where(keep, np.float16(1.0 / (1.0 - DROPOUT_P)),
                   np.float16(0.0)).astype(np.float16)
    _NC_CACHE["msk"] = msk
    return msk


def kernel(C, Q, W):
    C = np.ascontiguousarray(C, dtype=np.float32)
    Q = np.ascontiguousarray(Q, dtype=np.float32)
    W = np.ascontiguousarray(W, dtype=np.float32)
    msk = _gen_mask()
    eye = np.eye(D, dtype=np.float32)
    nc = _get_nc()
    in_maps = []
    for i in range(NCORES):
        sl = slice(i * BPC, (i + 1) * BPC)
        in_maps.append({"C": C[sl], "Q": Q[sl], "W": W[sl], "MSK": msk[sl],
                        "EYE": eye,
                        "ONES": np.ones((D, D), dtype=ml_dtypes.bfloat16)})
    res = bass_utils.run_bass_kernel_spmd(nc, in_maps, core_ids=list(range(NCORES)))
    out = np.concatenate([res.results[i]["OUT"] for i in range(NCORES)], axis=0)
    return out.astype(np.float32)


# revision 19
# speedup vs baseline: 1.0265x; 1.0021x over previous
"""Trainium2 Bass kernel for nn_CQAttention (B=24, D=128, N=M=2048), 8 cores.

Data-parallel: 3 batches per core. Per batch (all layouts partition-major):
  Qp[d,m] = Wm[d]*Q[d,m] + Wc[d]                       (DVE, f32r out)
  RT[m,n] = sum_d Qp[d,m] C[d,n] = st[n,m] + sc[n]     (PE, f32r, 1cyc/row)
  sq[m]   = sum_d Wq[d] Q[d,m]                          (PE, fp32, column MMs)
  stripe[m,n] = exp(RT + sq[m] - 60)   bf16, transient per m-tile (ACT)
  rs[m] = rowsum(stripe)  via accum_out;  rsr = 1/rs
  EgS[m,n] = stripe * rsr   fp16  (= S2^T, persistent 2048x2048)
  Aun[d,n] = sum_m QT[m,d] stripe[m,n]                  (PE, fp16 x bf16, phase 1)
  Z1A[p,n] = sum_m rs[m] * EgS[m,n]  (= colsum of EgT)  (PE, RSrep bf16 x fp16)
  EgSR[m,k] = rs[m] * EgS[m,k]  (~EgT), per k-chunk stripe (DVE)
  P[n,k]  = sum_m EgS[m,n] EgSR[m,k]  (symmetric)       (PE, fp16 x bf16)
  Btun[d,k] = sum_n CT[n,d] P[n,k]                      (PE, f32r x f32r)
  out = [C, Aun*rZ, C*Aun*rZ, C*Btun*rZ] * mask         (DVE)
where rZ = 1/Z1A broadcast (Z1A never ~0: worst row max of S is ~2.9 -> e^-57).
Dropout mask (jax key 42) generated host-side in a subprocess, applied on-device.
"""
import os
import subprocess
import sys
import tempfile
from contextlib import ExitStack

import numpy as np
import ml_dtypes

import concourse.bass as bass
import concourse.tile as tile
from concourse import bacc, mybir
from concourse import bass_utils
from concourse import bass_isa

B, D, N = 24, 128, 2048
NCORES, BPC = 8, 3
NT = N // 128          # 16 column-tiles of 128
NCH = N // 512         # 4 chunks of 512
KSH = 60.0             # static softmax shift
DROPOUT_P = 0.1

f32 = mybir.dt.float32
f32r = mybir.dt.float32r
bf16 = mybir.dt.bfloat16
f16 = mybir.dt.float16
Alu = mybir.AluOpType
ExpF = mybir.ActivationFunctionType.Exp


def _body(nc, tc, ctx, C_d, Q_d, W_d, MSK_d, EYE_d, ONES_d, OUT_d, DBG=None):
    const = ctx.enter_context(tc.tile_pool(name="const", bufs=1))
    big = ctx.enter_context(tc.tile_pool(name="big", bufs=1))
    stg = ctx.enter_context(tc.tile_pool(name="stg", bufs=40))
    io = ctx.enter_context(tc.tile_pool(name="io", bufs=1))
    io2 = ctx.enter_context(tc.tile_pool(name="io2", bufs=2))
    small = ctx.enter_context(tc.tile_pool(name="small", bufs=2))
    pnp = ctx.enter_context(tc.tile_pool(name="pnp", bufs=5))
    outp = ctx.enter_context(tc.tile_pool(name="outp", bufs=3))
    mskp = ctx.enter_context(tc.tile_pool(name="mskp", bufs=1))

    EYEt = const.tile([D, D], f32, tag="eye")
    nc.sync.dma_start(EYEt[:], EYE_d)
    ONESb = const.tile([D, D], bf16, tag="ones")
    nc.sync.dma_start(ONESb[:], ONES_d)

    for b in range(BPC):
        ctx.enter_context(nc.named_scope(f"b{b}"))
        # ---------------- prologue ----------------
        Csb = io2.tile([D, N], f32, tag="Csb")
        Qsb = io.tile([D, N], f32, tag="Qsb")
        nc.sync.dma_start(Csb[:], C_d[b])
        nc.sync.dma_start(Qsb[:], Q_d[b])
        Wt = small.tile([D, 3], f32, tag="Wt")      # cols: Wq, Wc, Wm
        nc.sync.dma_start(Wt[:], W_d[b, 0].rearrange("(t d) -> d t", d=D))

        Qp = io.tile([D, N], f32r, tag="s2")        # shared with Ablk
        nc.vector.tensor_scalar(Qp[:], Qsb[:], Wt[:, 2:3], Wt[:, 1:2],
                                Alu.mult, Alu.add)
        Cr = io.tile([D, N], f32r, tag="s3")        # shared with Btb
        nc.vector.tensor_copy(Cr[:], Csb[:])

        EgT = big.tile([D, NT * N], bf16, tag="EgT")   # 64 KB/partition
        rs = small.tile([D, NT], f32, tag="rs")
        rsr = small.tile([D, NT], f32, tag="rsr")

        with tc.tile_pool(name=f"psp_{b}", bufs=1, space="PSUM") as psp:
            # sq[m] as columns of [128, 16] (one zero-region group)
            sqp = psp.tile([D, NT], f32, tag="pro")
            for mt in range(NT):
                nc.tensor.matmul(sqp[:, mt:mt + 1],
                                 Qsb[:, mt * 128:(mt + 1) * 128], Wt[:, 0:1],
                                 start=(mt == 0), stop=(mt == NT - 1))
            sqK = small.tile([D, NT], f32, tag="sqK")
            nc.vector.tensor_scalar_add(sqK[:], sqp[:], -KSH)

            # transposes into one 4-bank tile each; groups of 4 slices per bank
            QTt = io.tile([D, N], f16, tag="QT")
            CTt = io.tile([D, N], f32r, tag="CT")
            tpq = psp.tile([D, N], f32, tag="pro")
            for mt in range(NT):
                sl = slice(mt * 128, (mt + 1) * 128)
                nc.tensor.matmul(tpq[:, sl], Qsb[:, sl], EYEt[:],
                                 is_transpose=True,
                                 start=(mt % 4 == 0), stop=(mt % 4 == 3))
            nc.scalar.copy(QTt[:], tpq[:])
            tpc = psp.tile([D, N], f32, tag="pro")
            for mt in range(NT):
                sl = slice(mt * 128, (mt + 1) * 128)
                nc.tensor.matmul(tpc[:, sl], Csb[:, sl], EYEt[:],
                                 is_transpose=True,
                                 start=(mt % 4 == 0), stop=(mt % 4 == 3))
            nc.vector.tensor_copy(CTt[:], tpc[:])

        # ------- phase 1: RT -> exp into EgT (half-tiles); A accumulation ----
        with tc.tile_pool(name=f"psr_{b}", bufs=2, space="PSUM") as psr, \
             tc.tile_pool(name=f"psa_{b}", bufs=1, space="PSUM") as psa:
            acc_a = psa.tile([D, N], f32, tag="acca")
            for mt in range(NT):
                msl = slice(mt * 128, (mt + 1) * 128)
                rsh = small.tile([D, 2], f32, tag="rsh")
                for h in range(2):
                    rth = psr.tile([D, 1024], f32, tag="rth")
                    for kcl in range(2):
                        kc = h * 2 + kcl
                        nc.tensor.matmul(rth[:, kcl * 512:(kcl + 1) * 512],
                                         Qp[:, msl],
                                         Cr[:, kc * 512:(kc + 1) * 512],
                                         start=True, stop=True)
                    egt_h = EgT[:, mt * N + h * 1024:mt * N + (h + 1) * 1024]
                    nc.scalar.activation(egt_h, rth[:], ExpF,
                                         bias=sqK[:, mt:mt + 1], scale=1.0,
                                         accum_out=rsh[:, h:h + 1])
                    for kcl in range(2):
                        kc = h * 2 + kcl
                        nc.tensor.matmul(
                            acc_a[:, kc * 512:(kc + 1) * 512], QTt[:, msl],
                            egt_h[:, kcl * 512:(kcl + 1) * 512],
                            start=(mt == 0), stop=(mt == NT - 1))
                nc.vector.tensor_add(rs[:, mt:mt + 1], rsh[:, 0:1], rsh[:, 1:2])
                nc.vector.reciprocal(rsr[:, mt:mt + 1], rs[:, mt:mt + 1])

            # ------- phase 2a: A evac (ACT); Z1 pre-pass on rth slots -------
            Asb = io.tile([D, N], f32, tag="s1")
            nc.scalar.copy(Asb[:], acc_a[:])
            rZ1b = io2.tile([D, N], f32, tag="rZ1b")
            for kc in range(NCH):
                ksl = slice(kc * 512, (kc + 1) * 512)
                z1c = psr.tile([D, 512], f32, tag="rth")
                for mt in range(NT):
                    nc.tensor.matmul(
                        z1c[:], ONESb[:],
                        EgT[:, mt * N + kc * 512:mt * N + kc * 512 + 512],
                        start=(mt == 0), stop=(mt == NT - 1))
                nc.vector.reciprocal(rZ1b[:, ksl], z1c[:])

        # ------- phase 2b: P (nt-outer) + Bt -------
        Btb = io.tile([D, N], f32, tag="Btb")

        def stage_est(nt):
            nsl0 = nt * 128
            tiles = []
            for mt in range(NT):
                est_t = stg.tile([D, D], f16, tag="egst")
                nc.vector.tensor_scalar_mul(
                    est_t[:], EgT[:, mt * N + nsl0:mt * N + nsl0 + 128],
                    rsr[:, mt:mt + 1])
                tiles.append(est_t)
            return tiles

        est_q = {0: stage_est(0), 1: stage_est(1)}
        with tc.tile_pool(name=f"pspc_{b}", bufs=4, space="PSUM") as pspc, \
             tc.tile_pool(name=f"psbt_{b}", bufs=1, space="PSUM") as psbt:
            btc = []
            for kc in range(NCH):
                btc_t = psbt.tile([D, 512], f32, tag=f"btc{kc}")
                btc.append(btc_t)
            for nt in range(NT):
                nsl0 = nt * 128
                est = est_q.pop(nt)
                if nt + 2 < NT:
                    est_q[nt + 2] = stage_est(nt + 2)
                pns = []
                for pr in range(2):
                    pch = []
                    for kcl in range(2):
                        pch_t = pspc.tile([D, 512], f32, tag="pc")
                        pch.append(pch_t)
                    for mt in range(NT):
                        for kcl in range(2):
                            kc = pr * 2 + kcl
                            nc.tensor.matmul(
                                pch[kcl][:], est[mt][:],
                                EgT[:, mt * N + kc * 512:mt * N + kc * 512 + 512],
                                start=(mt == 0), stop=(mt == NT - 1))
                    for kcl in range(2):
                        kc = pr * 2 + kcl
                        pn = pnp.tile([D, 512], f32r, tag="Pn")
                        nc.vector.tensor_copy(pn[:], pch[kcl][:])
                        pns.append((kc, pn))
                for kc, pn in pns:
                    nc.tensor.matmul(btc[kc][:], CTt[:, nsl0:nsl0 + 128],
                                     pn[:],
                                     start=(nt == 0), stop=(nt == NT - 1))
            for kc in range(NCH):
                kslb = slice(kc * 512, (kc + 1) * 512)
                nc.vector.tensor_mul(Btb[:, kslb], btc[kc][:], rZ1b[:, kslb])

        # ---------------- phase 3: outputs ----------------
        if DBG is not None and b == 0:
            nc.sync.dma_start(DBG["Wt"], Wt[:])
            nc.sync.dma_start(DBG["sqK"], sqK[:])
            nc.sync.dma_start(DBG["rs"], rs[:])
            nc.sync.dma_start(DBG["rZ1b"], rZ1b[:])
            nc.sync.dma_start(DBG["Asb"], Asb[:])
            nc.sync.dma_start(DBG["QTt"], QTt[:])
            nc.sync.dma_start(DBG["CTt"], CTt[:].bitcast(mybir.dt.float32))
            nc.sync.dma_start(DBG["EgS0"], EgT[:, 0:N])
        m0 = mskp.tile([D, N], f16, tag="msk")
        nc.sync.dma_start(m0[:], MSK_d[b, 0:D])
        o0 = outp.tile([D, N], f32, tag="ob")
        nc.vector.tensor_mul(o0[:], Csb[:], m0[:])
        nc.sync.dma_start(OUT_d[b, 0:D], o0[:])

        Ablk = Asb
        nc.vector.tensor_mul(Ablk[:], Asb[:], rZ1b[:])
        m1 = mskp.tile([D, N], f16, tag="msk")
        nc.sync.dma_start(m1[:], MSK_d[b, D:2 * D])
        o1 = outp.tile([D, N], f32, tag="ob")
        nc.vector.tensor_mul(o1[:], Ablk[:], m1[:])
        nc.sync.dma_start(OUT_d[b, D:2 * D], o1[:])

        m2 = mskp.tile([D, N], f16, tag="msk")
        nc.sync.dma_start(m2[:], MSK_d[b, 2 * D:3 * D])
        o2 = outp.tile([D, N], f32, tag="ob")
        nc.vector.tensor_mul(o2[:], Csb[:], Ablk[:])
        nc.vector.tensor_mul(o2[:], o2[:], m2[:])
        nc.sync.dma_start(OUT_d[b, 2 * D:3 * D], o2[:])

        m3 = mskp.tile([D, N], f16, tag="msk")
        nc.sync.dma_start(m3[:], MSK_d[b, 3 * D:4 * D])
        o3 = outp.tile([D, N], f32, tag="ob")
        nc.vector.tensor_mul(o3[:], Csb[:], Btb[:])
        nc.vector.tensor_mul(o3[:], o3[:], m3[:])
        nc.sync.dma_start(OUT_d[b, 3 * D:4 * D], o3[:])


_NC_CACHE = {}


def _get_nc():
    if "nc" not in _NC_CACHE:
        nc = bacc.Bacc("TRN2", target_bir_lowering=False, debug=False,
                       num_devices=NCORES)
        C_d = nc.dram_tensor("C", [BPC, D, N], f32, kind="ExternalInput").ap()
        Q_d = nc.dram_tensor("Q", [BPC, D, N], f32, kind="ExternalInput").ap()
        W_d = nc.dram_tensor("W", [BPC, 1, 3 * D], f32, kind="ExternalInput").ap()
        MSK_d = nc.dram_tensor("MSK", [BPC, 4 * D, N], f16,
                               kind="ExternalInput").ap()
        EYE_d = nc.dram_tensor("EYE", [D, D], f32, kind="ExternalInput").ap()
        ONES_d = nc.dram_tensor("ONES", [D, D], bf16, kind="ExternalInput").ap()
        OUT_d = nc.dram_tensor("OUT", [BPC, 4 * D, N], f32,
                               kind="ExternalOutput").ap()
        with tile.TileContext(nc) as tc, ExitStack() as ctx:
            _body(nc, tc, ctx, C_d, Q_d, W_d, MSK_d, EYE_d, OUT_d)
        nc.compile()
        _NC_CACHE["nc"] = nc
    return _NC_CACHE["nc"]


def _gen_mask():
    """Dropout keep-mask from jax key 42, via a CPU-jax subprocess."""
    if "msk" in _NC_CACHE:
        return _NC_CACHE["msk"]
    code = (
        "import jax, numpy as np, sys\n"
        "jax.config.update('jax_platforms', 'cpu')\n"
        "keep = np.asarray(jax.random.bernoulli(jax.random.key(42), "
        f"{1.0 - DROPOUT_P}, ({B}, {4 * D}, {N})))\n"
        "np.save(sys.argv[1], keep)\n"
    )
    with tempfile.NamedTemporaryFile(suffix=".npy", delete=False) as f:
        path = f.name
    env = dict(os.environ)
    env["JAX_PLATFORMS"] = "cpu"
    subprocess.run([sys.executable, "-c", code, path], env=env, check=True,
                   capture_output=True)
    keep = np.load(path)
    os.unlink(path)
    msk = np.where(keep, np.float16(1.0 / (1.0 - DROPOUT_P)),
                   np.float16(0.0)).astype(np.float16)
    _NC_CACHE["msk"] = msk
    return msk


def kernel(C, Q, W):
    C = np.ascontiguousarray(C, dtype=np.float32)
    Q = np.ascontiguousarray(Q, dtype=np.float32)
    W = np.ascontiguousarray(W, dtype=np.float32)
    msk = _gen_mask()
    eye = np.eye(D, dtype=np.float32)
    nc = _get_nc()
    in_maps = []
    for i in range(NCORES):
        sl = slice(i * BPC, (i + 1) * BPC)
        in_maps.append({"C": C[sl], "Q": Q[sl], "W": W[sl], "MSK": msk[sl],
                        "EYE": eye,
                        "ONES": np.ones((D, D), dtype=ml_dtypes.bfloat16)})
    res = bass_utils.run_bass_kernel_spmd(nc, in_maps, core_ids=list(range(NCORES)))
    out = np.concatenate([res.results[i]["OUT"] for i in range(NCORES)], axis=0)
    return out.astype(np.float32)
